# revision 1
# baseline (speedup 1.0000x reference)
"""Trainium2 Bass kernel for nn_Encoder2 (KAN encoder forward).

Reference computation (per row n of x, IN=128, OUT=64):
  z      = silu(x) @ (scale_base*mask) + einsum('nik,iok->no', B(x), coef*(scale_sp*mask))
  z      = z / max(||z||_2, 1e-12) * 0.8
  x_lin  = x @ W1.T
where B is the cubic B-spline basis (k=3) on a per-dim uniform extended grid.

Key algebraic transformation: on a uniform grid the B-splines are cardinal,
  B_j(t) = (1/6) * sum_{p=0..4} w_p * relu(t - j - p)^3,  w = [1,-4,6,-4,1]
with t = (x - g0)/h clamped to [0, 11].  So
  einsum(B, Csp) = V @ D
where V_q[n,i] = relu(min(x[n,i] - (g0_i + q h_i), (11-q) h_i))^3 (q = 0..10;
q=11 is identically zero after the clamp) and D is a host-side fold of the
w-stencil and 1/h^3 into coef*(scale_sp*mask).  V_q is computed on-device by
one fused custom DVE instruction per plane; the contraction runs on the
TensorEngine with K = 11*128 (+ silu and x_lin chunks).

Sharding: data-parallel over rows across 8 NeuronCores, parameters
replicated; no cross-core communication (edge_index is unused by the
forward).
"""

import numpy as np
from dataclasses import dataclass, field
from contextlib import ExitStack

import concourse.bass as bass
import concourse.mybir as mybir
import concourse.tile as tile
from concourse import dve_ops as _dvo
from concourse.dve_spec import Spec, Src0, C0, C1, relu, sq, minn, lower as _dve_lower
from concourse.dve_uop import (
    DveOpSpec, UopConfig, UopDpConfig, AluOp, AluInp, InpSel, OutSel, OutPath,
    Trigger, DelayInp, ENABLE,
)
from concourse.bass_utils import run_bass_kernel_spmd

AF = mybir.ActivationFunctionType
F32 = mybir.dt.float32

SILU_FUNC = AF.Silu  # sim_test swaps this (CoreSim has no Silu)

M_CORES = 8
TN = 896            # rows per pipeline tile (7 * 128)
KB = TN // 128      # row-blocks of 128 per tile
SUB = 448           # matmul free-dim subtile (<=512 fp32 PSUM bank)
NQ = 11             # truncated-power feature planes per input dim
NW = 13             # stationary chunks: 11 spline + silu + linear


# --------------------------------------------------------------------------
# custom DVE op: out = relu(min(in0 - s0, s1))^3, s0/s1 per-partition scalars
# --------------------------------------------------------------------------
_CUBE = None


def _get_cube_op():
    global _CUBE
    if _CUBE is not None:
        return _CUBE
    name = "CUBE_SHIFT_CLAMP_ANT"
    for op in _dvo.OPS:
        if op.name == name:
            _CUBE = op
            return op
    r = relu(minn(Src0 - C0, C1))
    spec = Spec(
        body=sq(r) * r,
        reference=lambda in0, in1, s0, s1, imm2: (
            np.maximum(np.minimum(in0.astype(np.float32) - s0, s1), 0.0) ** 3
        ).astype(np.float32),
    )
    row = _dvo._CUSTOM_DVE_ROW_BASE + len(_dvo.OPS)
    assert row < 0x20
    shas = {}
    for ver in ("v3", "v4"):
        try:
            uops = _dve_lower(spec, ver=ver)
            shas[ver] = DveOpSpec(name=name, opcode=row, uops=uops, rd1_en=False).sha(ver)
        except Exception:
            pass
    assert "v3" in shas, "CUBE op failed to lower for v3/TRN2"
    op = _dvo.DveOp(name, spec, subdim=False, uops_sha=shas)
    _dvo.OPS.append(op)
    _dvo.CUSTOM_DVE_SPECS[name] = spec
    _dvo._SUB_OPCODE_FOR_NAME[name] = row
    _CUBE = op
    return op


# --------------------------------------------------------------------------
# hand-authored dual cube op: processes element PAIRS.
#   out[p, n, 0] = relu(in0[p, n, 0] - s0[p])^3
#   out[p, n, 1] = relu(in0[p, n, 1] - s1[p])^3
# Call with in0 = [P, N, 2] (inner step 0: each value read twice) and
# out = [P, N, 2] (inner step = plane stride: results de-interleaved).
# The 2X_2PORT uop computes both cubes per cycle (2 reads / 2 writes);
# the REGULAR variant ping-pongs two 4-stage uops by element parity.
# --------------------------------------------------------------------------
# Hand-authored dual-output uop reached HW but produced wrong values
# (2X_2PORT wiring unverified) — keep disabled; single-cube Spec-DSL op
# is the validated path.
USE_DUAL = False
_DUAL = None


def _cube_chain(dp, s, src_a, const_a, zero_lane, cap_lane):
    """Wire stages s..s+3 of `dp` as cube(relu(A - C)); result in s+3 ALU."""
    dp[s].enable_alu(AluOp.SUBTRACT, src_a, const_a)
    dp[s + 1].enable_alu(AluOp.MAX, AluInp.PREV_ALU_OUT,
                         AluInp(int(AluInp.PREV_DELAY_0) + zero_lane))
    dp[s + 2].enable_alu(AluOp.MULTIPLY, AluInp.PREV_ALU_OUT,
                         AluInp.PREV_ALU_OUT)
    dp[s + 2].enable_delay_from_src(DelayInp.PREV_ALU_OUT, cap_lane)
    dp[s + 3].enable_alu(AluOp.MULTIPLY, AluInp.PREV_ALU_OUT,
                         AluInp(int(AluInp.PREV_DELAY_0) + cap_lane))


def _build_dual_uops():
    """2X_2PORT uop: lanes: d0=SRC_0, d1=CONST_0, d2=ZERO, d3=SRC_1,
    d4=CONST_1.  Chain A in s0-3 (capture r0 in d0, V0 in d1), chain B in
    s4-7 (capture r1 in d3).  WR0_LO <- delay1 (V0), WR1_LO <- ALU (V1)."""
    u = UopConfig()
    u.enable_input(InpSel.SRC_0, 1)
    u.enable_input(InpSel.CONST_0, 2)
    u.enable_input(InpSel.ZERO, 3)
    u.enable_input(InpSel.SRC_1, 4)
    u.enable_input(InpSel.CONST_1, 5)
    dp = u.datapath_config
    # chain A: V0 = cube(relu(d0 - d1)); keep d2(zero) d3(src1) d4(C1) alive
    dp[0].enable_alu(AluOp.SUBTRACT, AluInp.PREV_DELAY_0, AluInp.PREV_DELAY_1)
    dp[0].pass_through_delay(2, 3, 4)
    dp[1].enable_alu(AluOp.MAX, AluInp.PREV_ALU_OUT, AluInp.PREV_DELAY_2)
    dp[1].pass_through_delay(2, 3, 4)
    dp[2].enable_alu(AluOp.MULTIPLY, AluInp.PREV_ALU_OUT, AluInp.PREV_ALU_OUT)
    dp[2].enable_delay_from_src(DelayInp.PREV_ALU_OUT, 0)  # r0
    dp[2].pass_through_delay(2, 3, 4)
    dp[3].enable_alu(AluOp.MULTIPLY, AluInp.PREV_ALU_OUT, AluInp.PREV_DELAY_0)
    dp[3].pass_through_delay(2, 3, 4)
    # chain B: V1 = cube(relu(d3 - d4)); capture V0 into d1 at s4
    dp[4].enable_alu(AluOp.SUBTRACT, AluInp.PREV_DELAY_3, AluInp.PREV_DELAY_4)
    dp[4].enable_delay_from_src(DelayInp.PREV_ALU_OUT, 1)  # V0
    dp[4].pass_through_delay(2)
    dp[5].enable_alu(AluOp.MAX, AluInp.PREV_ALU_OUT, AluInp.PREV_DELAY_2)
    dp[5].pass_through_delay(1)
    dp[6].enable_alu(AluOp.MULTIPLY, AluInp.PREV_ALU_OUT, AluInp.PREV_ALU_OUT)
    dp[6].enable_delay_from_src(DelayInp.PREV_ALU_OUT, 3)  # r1
    dp[6].pass_through_delay(1)
    dp[7].enable_alu(AluOp.MULTIPLY, AluInp.PREV_ALU_OUT, AluInp.PREV_DELAY_3)
    dp[7].pass_through_delay(1)
    u.enable_output(OutSel.DELAY_1, OutPath.WR0_LO)
    u.enable_output(OutSel.ALU_OUT, OutPath.WR1_LO)
    u.require_inp0 = ENABLE
    u.require_inp1 = ENABLE
    u.trigger = (Trigger.SRC_TENSOR_DONE, Trigger.NONE, Trigger.NONE)
    u.next_uop = (0, 0, 0)

    def one_x(const_sel, nxt):
        """REGULAR variant: one element per cycle, alternating C0/C1."""
        v = UopConfig()
        v.enable_input(InpSel.SRC_0, 1)
        v.enable_input(const_sel, 2)
        v.enable_input(InpSel.ZERO, 3)
        dvp = v.datapath_config
        dvp[0].enable_alu(AluOp.SUBTRACT, AluInp.PREV_DELAY_0, AluInp.PREV_DELAY_1)
        dvp[0].pass_through_delay(2)
        dvp[1].enable_alu(AluOp.MAX, AluInp.PREV_ALU_OUT, AluInp.PREV_DELAY_2)
        dvp[2].enable_alu(AluOp.MULTIPLY, AluInp.PREV_ALU_OUT, AluInp.PREV_ALU_OUT)
        dvp[2].enable_delay_from_src(DelayInp.PREV_ALU_OUT, 0)
        dvp[3].enable_alu(AluOp.MULTIPLY, AluInp.PREV_ALU_OUT, AluInp.PREV_DELAY_0)
        for s in range(4, 8):
            dvp[s].pass_through_alu()
        v.enable_output(OutSel.ALU_OUT, OutPath.WR0_LO)
        v.require_inp0 = ENABLE
        v.repeat_count = 1
        v.trigger = (Trigger.SRC_TENSOR_DONE, Trigger.COUNT, Trigger.NONE)
        v.next_uop = (0, nxt, 0)
        return v

    uops_1x = [one_x(InpSel.CONST_0, 1), one_x(InpSel.CONST_1, 2),
               one_x(InpSel.CONST_0, 1)]
    # table-gen requires equal uop counts across variants; pad with
    # unreachable copies (steady uop never chains past index 0)
    import copy as _copy
    uops_2x2p = [u, _copy.deepcopy(u), _copy.deepcopy(u)]
    return uops_1x, uops_2x2p


@dataclass(frozen=True)
class _HandDveOp(_dvo.DveOp):
    hand: object = None

    def compile(self, ver):
        assert ver == "v3", f"dual cube op only authored for v3, got {ver}"
        return self.hand


def _get_dual_op():
    global _DUAL
    if _DUAL is not None:
        return _DUAL
    name = "DUAL_CUBE_ANT"
    for op in _dvo.OPS:
        if op.name == name:
            _DUAL = op
            return op

    def _ref(in0, in1, s0, s1, imm2):
        a = in0.astype(np.float32)
        e = np.maximum(a[..., 0] - s0, 0.0) ** 3
        o = np.maximum(a[..., 1] - s1, 0.0) ** 3
        return np.stack([e, o], axis=-1).astype(np.float32)

    r = relu(Src0 - C0)
    spec = Spec(body=sq(r) * r, reference=_ref)
    row = _dvo._CUSTOM_DVE_ROW_BASE + len(_dvo.OPS)
    assert row < 0x20
    uops_1x, uops_2x2p = _build_dual_uops()
    hand = DveOpSpec(
        name=name, opcode=row, uops=uops_1x,
        uops_2x=uops_2x2p, uops_2x_2p=uops_2x2p, uops_4x=None,
        perf_max=2, rd1_en=False,
    )
    op = _HandDveOp(name, spec, subdim=False, uops_sha={}, hand=hand)
    _dvo.OPS.append(op)
    _dvo.CUSTOM_DVE_SPECS[name] = spec
    _dvo._SUB_OPCODE_FOR_NAME[name] = row
    _DUAL = op
    return op


# --------------------------------------------------------------------------
# device program (SPMD, one core's shard of rows)
# --------------------------------------------------------------------------
_PROGRAMS = {}


def _build_program(R):
    """Bass program processing R (multiple of TN) rows of x."""
    nt = R // TN
    cube = _get_cube_op()

    nc = bass.Bass(trn_type="TRN2")
    xs = nc.declare_dram_parameter("xs", [R, 128], F32, isOutput=False)
    wstack = nc.declare_dram_parameter("wstack", [NW, 128, 64], F32, isOutput=False)
    c0s = nc.declare_dram_parameter("c0s", [128, 16], F32, isOutput=False)
    c1s = nc.declare_dram_parameter("c1s", [128, 16], F32, isOutput=False)
    ident = nc.declare_dram_parameter("ident", [128, 128], F32, isOutput=False)
    z_out = nc.declare_dram_parameter("z_out", [R, 64], F32, isOutput=True)
    xl_out = nc.declare_dram_parameter("xl_out", [R, 64], F32, isOutput=True)

    xsv = xs[:].rearrange("(t k p) i -> t p k i", k=KB, p=128)
    zv = z_out[:].rearrange("(t k p) o -> t p k o", k=KB, p=128)
    xlv = xl_out[:].rearrange("(t k p) o -> t p k o", k=KB, p=128)

    with tile.TileContext(nc) as tc:
        with ExitStack() as ctx:
            const = ctx.enter_context(tc.tile_pool(name="const", bufs=1))
            p_xin = ctx.enter_context(tc.tile_pool(name="xin", bufs=2))
            p_xt = ctx.enter_context(tc.tile_pool(name="xt", bufs=2))
            p_silu = ctx.enter_context(tc.tile_pool(name="silu", bufs=2))
            p_v = ctx.enter_context(tc.tile_pool(name="v", bufs=2))
            p_zsb = ctx.enter_context(tc.tile_pool(name="zsb", bufs=2))
            p_fin = ctx.enter_context(tc.tile_pool(name="fin", bufs=2))
            p_small = ctx.enter_context(tc.tile_pool(name="small", bufs=2))
            ps_xt = ctx.enter_context(tc.tile_pool(name="ps_xt", bufs=2, space="PSUM"))
            ps_z = ctx.enter_context(tc.tile_pool(name="ps_z", bufs=4, space="PSUM"))
            ps_zt = ctx.enter_context(tc.tile_pool(name="ps_zt", bufs=1, space="PSUM"))
            ps_xl = ctx.enter_context(tc.tile_pool(name="ps_xl", bufs=1, space="PSUM"))

            wsb = const.tile([128, NW, 64], F32)
            nc.sync.dma_start(out=wsb[:], in_=wstack[:].rearrange("w p o -> p w o"))
            c0sb = const.tile([128, 16], F32)
            nc.sync.dma_start(out=c0sb[:], in_=c0s[:])
            c1sb = const.tile([128, 16], F32)
            nc.sync.dma_start(out=c1sb[:], in_=c1s[:])
            idsb = const.tile([128, 128], F32)
            nc.sync.dma_start(out=idsb[:], in_=ident[:])

            # Warm-up touches: each const lands in one engine's observed
            # vector clock via a single-wait instruction, so steady-state
            # PE/DVE instructions never need >1 sync wait (ISA limit).
            warm_t = ps_xt.tile([128, 128], F32, tag="pxt")
            nc.tensor.transpose(warm_t[:], idsb[:], idsb[:])
            warm_m = ps_xt.tile([128, 128], F32, tag="pxt")
            nc.tensor.matmul(
                warm_m[:64, :16], wsb[:, 0, :], idsb[:, 0:16],
                start=True, stop=True,
            )
            warm_v = p_small.tile([128, 16], F32, tag="warm")
            nc.vector.memset(warm_v[:], 0.0)  # absorbs preamble dep
            nc.vector.tensor_scalar_max(warm_v[:], c0sb[:], 0.0)
            nc.vector.tensor_scalar_max(warm_v[:], c1sb[:], 0.0)

            for it in range(nt):
                # ---- load x rows (natural layout), transpose to [i, n] ----
                xin = p_xin.tile([128, KB, 128], F32)
                nc.sync.dma_start(out=xin[:], in_=xsv[it])
                xt = p_xt.tile([128, TN], F32)
                # absorber: observe xt slot release on ACT before real writes
                nc.scalar.activation(xt[:1, 0:2], idsb[:1, 0:2], AF.Copy)
                for k in range(KB):
                    pxt = ps_xt.tile([128, 128], F32, tag="pxt")
                    nc.tensor.transpose(pxt[:], xin[:, k, :], idsb[:])
                    nc.scalar.activation(xt[:, k * 128:(k + 1) * 128], pxt[:], AF.Copy)

                # ---- elementwise features ----
                silu = p_silu.tile([128, TN], F32)
                nc.scalar.activation(silu[:], xt[:], SILU_FUNC)
                v = p_v.tile([128, NQ + 1, TN], F32)
                # absorber: observe v slot release on DVE (1 wait) so cube
                # ops carry only their input dep
                nc.vector.memset(v[:1, 0, 0:1], 0.0)
                if USE_DUAL:
                    dual = _get_dual_op()
                    # pre-clamp x at the top knot (c1s col 0 = g0 + 11h)
                    xtc = p_xt.tile([128, TN], F32, tag="xtc")
                    nc.vector.tensor_scalar_min(xtc[:], xt[:], c1sb[:, 15:16])
                    src = xtc[:]
                    pair_in = bass.AP(
                        tensor=src.tensor, offset=src.offset,
                        ap=[src.ap[0], src.ap[1], [0, 2]],
                    )
                    for j in range((NQ + 1) // 2):
                        dst = v[:, 2 * j, :]
                        pair_out = bass.AP(
                            tensor=dst.tensor, offset=dst.offset,
                            ap=[dst.ap[0], dst.ap[1], [TN, 2]],
                        )
                        bi = nc.vector._custom_dve(
                            dual, out=pair_out, in0=pair_in,
                            s0=c0sb[:, 2 * j:2 * j + 1],
                            s1=c0sb[:, 2 * j + 1:2 * j + 2],
                        )
                        bi.ins.perf_max = 2
                else:
                    for q in range(NQ):
                        nc.vector._custom_dve(
                            cube, out=v[:, q, :], in0=xt[:],
                            s0=c0sb[:, q:q + 1], s1=c1sb[:, q:q + 1],
                        )

                # ---- z = V @ D + silu @ sb   (PSUM accumulate over 12 chunks)
                # q outer / s inner: one LDWEIGHTS per chunk feeds two
                # back-to-back N=448 matmuls, halving weight-load traffic
                pz0 = ps_z.tile([64, SUB], F32, tag="pz")
                pz1 = ps_z.tile([64, SUB], F32, tag="pz")
                pzs = [pz0, pz1]
                for q in range(NQ + 1):
                    for s in range(TN // SUB):
                        sl = slice(s * SUB, (s + 1) * SUB)
                        rhs = v[:, q, sl] if q < NQ else silu[:, sl]
                        nc.tensor.matmul(
                            pzs[s][:], wsb[:, q, :], rhs,
                            start=(q == 0), stop=(q == NQ),
                        )

                # ---- x_lin.T tiles: [n,o] = xt_chunk.T @ W1T ----
                pxl = ps_xl.tile([128, KB * 64], F32)
                for k in range(KB):
                    nc.tensor.matmul(
                        pxl[:, k * 64:(k + 1) * 64],
                        xt[:, k * 128:(k + 1) * 128], wsb[:, 12, :],
                        start=True, stop=True,
                    )
                xlfin = p_fin.tile([128, KB * 64], F32, tag="xlfin")
                # absorber: observe xlfin slot release (store DMA) on ACT
                nc.scalar.activation(xlfin[:1, 0:2], idsb[:1, 0:2], AF.Copy)
                nc.scalar.activation(xlfin[:], pxl[:], AF.Copy)
                nc.sync.dma_start(
                    out=xlv[it], in_=xlfin[:].rearrange("p (k o) -> p k o", k=KB)
                )

                # ---- transpose z to row-major, normalize ----
                zsb = p_zsb.tile([64, TN], F32)
                for s in range(TN // SUB):
                    nc.scalar.activation(
                        zsb[:, s * SUB:(s + 1) * SUB], pzs[s][:], AF.Copy
                    )
                pzt = ps_zt.tile([128, KB * 64], F32)
                # absorber: normal matmul (2-wait capable via LDW+MM split)
                # observes pzt slot release before the transposes (1-wait cap)
                nc.tensor.matmul(
                    pzt[:64, 0:1], idsb[:, :64], idsb[:, 0:1],
                    start=True, stop=True,
                )
                for k in range(KB):
                    nc.tensor.transpose(
                        pzt[:, k * 64:(k + 1) * 64],
                        zsb[:, k * 128:(k + 1) * 128], idsb[:64, :64],
                    )
                sqscr = p_small.tile([128, 64], F32, tag="sqscr")
                ssum = p_small.tile([128, KB], F32, tag="ssum")
                for k in range(KB):
                    nc.scalar.activation(
                        sqscr[:], pzt[:, k * 64:(k + 1) * 64], AF.Square,
                        accum_out=ssum[:, k:k + 1],
                    )
                # norm/0.8 = sqrt(S/0.64); then clamped reciprocal
                snorm = p_small.tile([128, KB], F32, tag="snorm")
                nc.scalar.activation(snorm[:], ssum[:], AF.Sqrt, scale=1.5625)
                snc = p_small.tile([128, KB], F32, tag="snc")
                nc.vector.tensor_scalar_max(snc[:], snorm[:], 1.25e-12)
                rn = p_small.tile([128, KB], F32, tag="rn")
                nc.vector.reciprocal(rn[:], snc[:])
                zfin = p_fin.tile([128, KB, 64], F32, tag="zfin")
                # absorber: observe zfin slot release (store DMA) on DVE
                nc.vector.memset(zfin[:1, 0, 0:1], 0.0)
                for k in range(KB):
                    nc.vector.tensor_scalar_mul(
                        zfin[:, k, :], pzt[:, k * 64:(k + 1) * 64], rn[:, k:k + 1]
                    )
                nc.sync.dma_start(out=zv[it], in_=zfin[:])
    return nc


def _split_multi_waits(nc):
    """Legalize sync waits: TPB ISA structs carry one wait slot, and
    single-struct instructions (tensor_scalar, transposes, custom DVE,
    memset, ...) cannot be split by walrus.  Move all but one wait of any
    multi-wait instruction onto freshly inserted same-engine NOPs placed
    immediately before it (waits commute, so this is semantics-preserving)."""
    skip = ("InstEventSemaphore", "InstUnconditionalBranch",
            "InstCall", "InstISA")
    n_fix = 0
    for bb in nc.main_func.blocks:
        out = []
        for ins in bb.instructions:
            si = getattr(ins, "sync_info", None)
            if (type(ins).__name__ == "InstISA"
                    and getattr(ins, "op_name", "") == "EVENT_SEMAPHORE_RANGE_CLEAR"):
                # This short sequencer encoding is rejected by this walrus
                # build ("ISA wrong length"); replace with explicit
                # sem-wr-imm 0 updates (one event-sem each; walrus caps
                # EventSemaphore at <=1 update).
                d = ins.ant_dict
                waits = list(si.on_wait) if si else []
                for j, k in enumerate(range(d["range_first"], d["range_last"] + 1)):
                    ev = mybir.InstEventSemaphore(
                        name=f"{ins.name}-semclr{j}", engine=ins.engine
                    )
                    ev.sync_info = mybir.SyncInfo(
                        on_wait=waits[:2] if j == 0 else [],
                        on_update=[mybir.SyncUpdate(
                            sync_type="semaphore", id=k,
                            update_mode="sem-wr-imm", update_value=0,
                        )],
                    )
                    out.append(ev)
                n_fix += 1
                continue
            if (si is not None and len(si.on_wait) > 1
                    and type(ins).__name__ not in skip):
                # EventSemaphore carries <=2 waits; chain as many as needed.
                extra = list(si.on_wait)[1:]
                for j in range(0, len(extra), 2):
                    ev = mybir.InstEventSemaphore(
                        name=f"{ins.name}-wsplit{j}", engine=ins.engine
                    )
                    ev.sync_info = mybir.SyncInfo(
                        on_wait=extra[j:j + 2], on_update=[]
                    )
                    out.append(ev)
                    n_fix += 1
                ins.sync_info = mybir.SyncInfo(
                    on_wait=[si.on_wait[0]], on_update=list(si.on_update)
                )
            out.append(ins)
        bb.instructions = out
    return n_fix


def _get_program(R):
    if R not in _PROGRAMS:
        nc = _build_program(R)
        # Pack .instr bytes for InstISA subclasses (custom DVE ops) —
        # raw Bass doesn't run this pass; without it walrus sees empty
        # instr ("ISA wrong length").
        mybir.codegen_inst_isa_subclasses(nc)
        _split_multi_waits(nc)
        _PROGRAMS[R] = nc
    return _PROGRAMS[R]


# --------------------------------------------------------------------------
# host-side parameter preparation
# --------------------------------------------------------------------------
def _prep_params(W1, grid, coef, scale_base, scale_sp, mask):
    IN, OUT = W1.shape[1], W1.shape[0]
    grid = np.asarray(grid, np.float64)
    g0 = grid[:, 0]
    h = grid[:, 1] - grid[:, 0]
    if not np.allclose(np.diff(grid, axis=1), h[:, None], rtol=1e-4, atol=1e-6):
        raise NotImplementedError("non-uniform B-spline grid not supported")
    sp = np.asarray(scale_sp, np.float64) * np.asarray(mask, np.float64)
    sb = (np.asarray(scale_base, np.float64) * np.asarray(mask, np.float64))
    Csp = np.asarray(coef, np.float64) * sp[:, :, None]          # (IN, OUT, 8)
    w5 = np.array([1.0, -4.0, 6.0, -4.0, 1.0])
    D = np.zeros((IN, OUT, 12))
    for j in range(Csp.shape[2]):
        for p in range(5):
            D[:, :, j + p] += Csp[:, :, j] * w5[p]
    D = D[:, :, :NQ] / 6.0
    D /= (h ** 3)[:, None, None]          # device cubes are in x units
    wstack = np.zeros((NW, IN, OUT), np.float32)
    wstack[:NQ] = np.transpose(D, (2, 0, 1)).astype(np.float32)
    wstack[NQ] = sb.astype(np.float32)
    wstack[NQ + 1] = np.asarray(W1, np.float32).T
    qs = np.arange(NQ)
    c0 = np.zeros((IN, 16), np.float32)
    c1 = np.zeros((IN, 16), np.float32)
    c0[:, :NQ] = (g0[:, None] + qs[None, :] * h[:, None]).astype(np.float32)
    c0[:, NQ] = 1e30  # dummy 12th plane shift -> relu(..)^3 == 0
    c1[:, :NQ] = ((11 - qs)[None, :] * h[:, None]).astype(np.float32)
    c1[:, 15] = (g0 + 11.0 * h).astype(np.float32)  # clamp bound for dual path
    return wstack, c0, c1


# --------------------------------------------------------------------------
# public entry point
# --------------------------------------------------------------------------
def run(x, W1, grid, coef, scale_base, scale_sp, mask, edge_index=None, **run_kw):
    x = np.asarray(x, np.float32)
    N, IN = x.shape
    per = (N + M_CORES - 1) // M_CORES
    R = ((per + TN - 1) // TN) * TN
    wstack, c0, c1 = _prep_params(W1, grid, coef, scale_base, scale_sp, mask)
    eye = np.eye(128, dtype=np.float32)

    xp = np.zeros((M_CORES, R, IN), np.float32)
    flat = x.reshape(M_CORES, per, IN) if N == M_CORES * per else None
    if flat is not None:
        xp[:, :per] = flat
    else:
        for c in range(M_CORES):
            seg = x[c * per:(c + 1) * per]
            xp[c, :seg.shape[0]] = seg

    in_maps = [
        dict(xs=xp[c], wstack=wstack, c0s=c0, c1s=c1, ident=eye)
        for c in range(M_CORES)
    ]
    nc = _get_program(R)
    res = run_bass_kernel_spmd(nc, in_maps, list(range(M_CORES)), **run_kw)
    zs, xls = [], []
    n_left = N
    for c in range(M_CORES):
        take = min(per, n_left)
        zs.append(res.results[c]["z_out"][:take])
        xls.append(res.results[c]["xl_out"][:take])
        n_left -= take
    z = np.concatenate(zs, 0).astype(np.float32)
    xl = np.concatenate(xls, 0).astype(np.float32)
    return (z, xl), res


def kernel(x, W1, grid, coef, scale_base, scale_sp, mask, edge_index=None):
    (z, xl), _ = run(x, W1, grid, coef, scale_base, scale_sp, mask, edge_index)
    return z, xl

